# revision 39
# baseline (speedup 1.0000x reference)
"""Multi-head causal attention (B=4, S=2048, D=1024, H=16, HD=64) on 8 trn2 cores.

Sharding: batch x head-group. Core c handles batch b = c//2 and heads
g*8..(g+1)*8 where g = c%2 (512 projection dims). The host sums the two
partial output projections per batch and adds the bias.

All matmuls run in bf16 (fp32r lowers to fp32_mode=HIGH on HW at ~2.5
cycles/row; bf16 streams at 1 cycle/row), accumulation stays fp32 in PSUM.

Schedule notes (v2):
  - PSUM tags are split so filler matmuls (projections / output proj) no
    longer rotate through the score slots: scores keep true
    double-buffering (tag "sc", 2x2 banks), fillers use tag "fl" (2x1),
    ctx accumulators tag "cx" (2x1).
  - The P@V pipeline (pending deque) flows across head-pair boundaries:
    the trailing P@Vs of head-pair h drain during the first iterations of
    h+1, so the PE never waits for the last exps of a head-pair.
  - The softmax normalization broadcast (1/denominator across the 64
    head-dim partitions) runs on GpSimd (partition_broadcast) instead of
    K=1 PE matmuls.
  - Weight DMAs are split per-2-ktile pieces across idle rings (wv on
    scalar, wq on vector, wk on sync) so first projections start as the
    pieces land; wo/cmask ride gpsimd.
  - Output-projection DMA writes ride gpsimd (sw dge) instead of the
    Activation ring, keeping the exp engine clear.
  - Epilogue: 6 partial output chains (k=0..2) overlap the final
    normalize; only the k=3 links wait for the last head-pair.
"""

from collections import deque
from contextlib import ExitStack

import numpy as np

import concourse.bass as bass
import concourse.tile as tile
from concourse import bacc, mybir
from concourse.bass_utils import run_bass_kernel_spmd

F32 = mybir.dt.float32
BF16 = mybir.dt.bfloat16
AF = mybir.ActivationFunctionType

B, S, D, H = 4, 2048, 1024, 16
HD = D // H          # 64
SCALE = float(np.sqrt(HD))
NCORES = 8
G = 2                # head groups (cores per batch)
HPC = H // G         # heads per core = 8
CW = HPC * HD        # per-core projection width = 512
KO = D // 128        # 8 contraction subtiles
OT = CW // 128       # 4 projection out-tiles (head pairs)
QCH = 512            # q chunk
NQT = S // 128       # 16 kv tiles
NCH = S // QCH       # 4 q chunks


def _emit(nc):
    XTB = nc.dram_tensor("XTB", [128, KO, S], BF16, kind="ExternalInput").ap()
    WQT = nc.dram_tensor("WQT", [128, KO, OT, 128], BF16, kind="ExternalInput").ap()
    WKT = nc.dram_tensor("WKT", [128, KO, OT, 128], BF16, kind="ExternalInput").ap()
    WVT = nc.dram_tensor("WVT", [128, KO, OT, 128], BF16, kind="ExternalInput").ap()
    WOT = nc.dram_tensor("WOT", [128, OT, D], BF16, kind="ExternalInput").ap()
    CMASK = nc.dram_tensor("CMASK", [128, 128], BF16, kind="ExternalInput").ap()
    OUT = nc.dram_tensor("OUT", [S, D], BF16, kind="ExternalOutput").ap()

    with tile.TileContext(nc) as tc, ExitStack() as ctx, \
            nc.allow_low_precision(reason="bf16 attention pipeline"):
        consts = ctx.enter_context(tc.tile_pool(name="consts", bufs=1))
        xpool = ctx.enter_context(tc.tile_pool(name="xpool", bufs=2))
        qkv = ctx.enter_context(tc.tile_pool(name="qkv", bufs=1))
        ptp = ctx.enter_context(tc.tile_pool(name="ptp", bufs=12))
        rpool = ctx.enter_context(tc.tile_pool(name="rpool", bufs=6))
        cpool = ctx.enter_context(tc.tile_pool(name="cpool", bufs=2))
        cnp = ctx.enter_context(tc.tile_pool(name="cnp", bufs=4))
        opool = ctx.enter_context(tc.tile_pool(name="opool", bufs=6))
        ps = ctx.enter_context(tc.tile_pool(name="ps", bufs=2, space="PSUM"))

        wq = consts.tile([128, KO, OT, 128], BF16, tag="wq")
        wk = consts.tile([128, KO, OT, 128], BF16, tag="wk")
        wv = consts.tile([128, KO, OT, 128], BF16, tag="wv")
        wo = consts.tile([128, OT, D], BF16, tag="wo")
        cmask = consts.tile([128, 128], BF16, tag="cmask")

        qt = qkv.tile([128, OT, S], BF16, tag="qt")
        kt = qkv.tile([128, OT, S], BF16, tag="kt")
        # v natural: [kv, kvtile, hp, hh, 65] with a ones column at 64.
        v_sb = qkv.tile([128, NQT, OT, 2, HD + 1], BF16, tag="v")
        nc.vector.memset(v_sb[:, :, :, :, HD:HD + 1], 1.0)
        ones64 = consts.tile([128, HD], BF16, tag="ones64")
        nc.vector.memset(ones64[:], 1.0)

        # weight pieces split so projections start as they land (small first
        # pieces for the earliest matmuls, wq interleaved before the wv tail
        # so the q projection isn't starved); few enough issues that the
        # Activation queue clears before the first exp.
        for k2 in range(0, KO, 2):
            nc.scalar.dma_start(wv[:, k2:k2 + 2], WVT[:, k2:k2 + 2])
        for k4 in range(0, KO, 4):
            nc.scalar.dma_start(wq[:, k4:k4 + 4], WQT[:, k4:k4 + 4])
        nc.scalar.dma_start(wo[:], WOT[:])

        state = {}
        fillers = deque()
        pending = deque()  # (pt, s, i, hp, cx0, cx1, stop, cn)
        deferred = []

        def emit_xdma(c):
            xt = xpool.tile([128, KO, QCH], BF16, tag="xt", name=f"xt{c}")
            for k in range(KO):  # per-ktile pieces so proj can start early
                nc.sync.dma_start(xt[:, k, :], XTB[:, k, bass.ts(c, QCH)])
            state["xt", c] = xt
            if c == 0:
                # wk + cmask ride sync after chunk-0 x.
                nc.sync.dma_start(cmask[:], CMASK[:])
                for k2 in range(0, KO, 2):
                    nc.sync.dma_start(wk[:, k2:k2 + 2], WKT[:, k2:k2 + 2])

        def proj_item(c, w, dst, ot):
            def f():
                xt = state["xt", c]
                pp = ps.tile([128, QCH], F32, tag="fl", name="pp")
                for k in range(KO):
                    nc.tensor.matmul(pp[:], w[:, k, ot, :], xt[:, k, :],
                                     start=(k == 0), stop=(k == KO - 1))
                nc.vector.tensor_copy(dst[:, ot, bass.ts(c, QCH)], pp[:])
            return f

        def vdir_item(c, i):
            """V in natural [kv, outdim] layout directly: x-tile stationary,
            Wv moving - no PE transpose needed."""
            def f():
                xt = state["xt", c]
                pp = ps.tile([128, QCH], F32, tag="fl", name="vd")
                for k in range(KO):
                    nc.tensor.matmul(
                        pp[:], xt[:, k, (i % 4) * 128:(i % 4 + 1) * 128],
                        wv[:, k, :, :].rearrange("p o n -> p (o n)"),
                        start=(k == 0), stop=(k == KO - 1))
                nc.vector.tensor_copy(
                    v_sb[:, i, :, :, 0:HD],
                    pp[:].rearrange("p (o h d) -> p o h d", o=OT, h=2))
            return f

        def vdir_pair(c, i0, i1):
            """Two v tiles with the per-ktile matmuls interleaved, so the
            DMA-paced prologue streams weight/x pieces into the PE as they
            land instead of stalling one chain per piece."""
            def f():
                xt = state["xt", c]
                pp0 = ps.tile([128, QCH], F32, tag="fl", name="vp0")
                pp1 = ps.tile([128, QCH], F32, tag="fl", name="vp1")
                wvf = wv[:, :, :, :].rearrange("p k o n -> p k (o n)")
                for k in range(KO):
                    for i, pp in ((i0, pp0), (i1, pp1)):
                        nc.tensor.matmul(
                            pp[:], xt[:, k, (i % 4) * 128:(i % 4 + 1) * 128],
                            wvf[:, k, :],
                            start=(k == 0), stop=(k == KO - 1))
                for i, pp in ((i0, pp0), (i1, pp1)):
                    nc.vector.tensor_copy(
                        v_sb[:, i, :, :, 0:HD],
                        pp[:].rearrange("p (o h d) -> p o h d", o=OT, h=2))
            return f

        def po_item(j, t, dc):
            def f():
                cn = state["cn", j]
                po = ps.tile([128, QCH], F32, tag="fl", name="po")
                for k in range(OT):
                    nc.tensor.matmul(po[:], cn[:, k, bass.ts(t, 128)],
                                     wo[:, k, bass.ts(dc, QCH)],
                                     start=(k == 0), stop=(k == OT - 1))
                ob = opool.tile([128, QCH], BF16, tag="ob")
                nc.vector.tensor_copy(ob[:], po[:])
                # gpsimd ring: a data-dependent write issue must not sit in
                # front of the latency-critical norm DMAs (sync) or exps
                # (scalar) - in-order queues stall behind it.
                nc.gpsimd.dma_start(
                    OUT[(4 * j + t) * 128:(4 * j + t + 1) * 128,
                        bass.ts(dc, QCH)],
                    ob[:])
            return f

        def push_chunk_fillers(c):
            """proj+vtrans for chunk c (to drain during chunk c-1). V first
            so its transposes overlap the q/k projections in the prologue."""
            fillers.append(vdir_item(c, 4 * c))
            fillers.append(vdir_item(c, 4 * c + 1))
            fillers.append(proj_item(c, wq, qt, 0))
            fillers.append(proj_item(c, wk, kt, 0))
            fillers.append(vdir_item(c, 4 * c + 2))
            fillers.append(vdir_item(c, 4 * c + 3))
            for ot in range(1, OT):
                fillers.append(proj_item(c, wq, qt, ot))
                fillers.append(proj_item(c, wk, kt, ot))

        def norm_a(hp, cn, cx0, cx1, ring=None):
            """Copy ctx+denom to SBUF (frees the cx PSUM banks), compute
            1/d partition-parallel: the two [1,512] denominator rows are
            DMA-reshaped to [128,8], one cheap DVE reciprocal covers both
            heads, and the rows are DMA-restored for a gpsimd broadcast."""
            ring = ring or nc.sync
            ub0 = cpool.tile([HD + 1, QCH], F32, tag="ub0")
            ub1 = cpool.tile([HD + 1, QCH], F32, tag="ub1")
            nc.vector.tensor_copy(ub0[:], cx0[0:HD + 1, :])
            nc.vector.tensor_copy(ub1[:], cx1[0:HD + 1, :])
            dsq = rpool.tile([128, 8], F32, tag="dsq")
            ring.dma_start(dsq[:, 0:4], ub0[HD:HD + 1, :])
            ring.dma_start(dsq[:, 4:8], ub1[HD:HD + 1, :])
            dsr = rpool.tile([128, 8], BF16, tag="dsr")
            nc.vector.reciprocal(dsr[:], dsq[:])
            r0 = rpool.tile([1, QCH], BF16, tag="r0")
            r1 = rpool.tile([1, QCH], BF16, tag="r1")
            ring.dma_start(r0[:], dsr[:, 0:4])
            ring.dma_start(r1[:], dsr[:, 4:8])
            return ub0, ub1, r0, r1

        def norm_b(hp, cn, ub0, ub1, r0, r1, ring=None):
            ring2 = ring or nc.sync

            def f():
                rb0 = rpool.tile([HD, QCH], BF16, tag="rb0")
                rb1 = rpool.tile([HD, QCH], BF16, tag="rb1")
                nc.gpsimd.partition_broadcast(rb0[:], r0[:], channels=HD)
                nc.gpsimd.partition_broadcast(rb1[:], r1[:], channels=HD)
                ct1 = cpool.tile([HD, QCH], BF16, tag="ct1")
                nc.vector.tensor_mul(ct1[:], ub1[0:HD, :], rb1[:])
                ring2.dma_start(cn[HD:128, hp, :], ct1[:])
                nc.vector.tensor_mul(cn[0:HD, hp, :], ub0[0:HD, :], rb0[:])
            return f

        def norm_b_pe(cn, ub0, ub1, r0, r1):
            """Final head-pair: broadcast 1/d with K=1 PE matmuls (no gpsimd
            wakeup+serial-broadcast hop on the critical path)."""
            rbt1 = ps.tile([128, QCH], F32, tag="fl", name="rbt1")
            rbt0 = ps.tile([128, QCH], F32, tag="fl", name="rbt0")
            nc.tensor.matmul(rbt1[0:HD, :], ones64[0:1, 0:HD], r1[:],
                             start=True, stop=True)
            nc.tensor.matmul(rbt0[0:HD, :], ones64[0:1, 0:HD], r0[:],
                             start=True, stop=True)
            ct1 = cpool.tile([HD, QCH], BF16, tag="ct1")
            nc.vector.tensor_mul(ct1[:], ub1[0:HD, :], rbt1[0:HD, :])
            nc.scalar.dma_start(cn[HD:128, OT - 1, :], ct1[:])
            nc.vector.tensor_mul(cn[0:HD, OT - 1, :], ub0[0:HD, :],
                                 rbt0[0:HD, :])

        def pop_pv():
            ppt, s, pi, hp, cx0, cx1, stop, cn, final = pending.popleft()
            for hh, cx in ((0, cx0), (1, cx1)):
                nc.tensor.matmul(cx[0:HD + 1, s:QCH],
                                 v_sb[:, pi, hp, hh, :],
                                 ppt[:, hh, s:QCH],
                                 start=(pi == 0), stop=stop)
            if stop:
                if final:
                    # latency-critical: recips on the empty scalar ring; the
                    # PE broadcast half is emitted later (state["fnormb"]).
                    ub0, ub1, r0, r1 = norm_a(hp, cn, cx0, cx1,
                                              ring=nc.scalar)
                    state["fnormb"] = lambda: norm_b_pe(cn, ub0, ub1, r0, r1)
                else:
                    deferred.append(norm_b(
                        hp, cn, *norm_a(hp, cn, cx0, cx1)))

        def attention_chunk(j):
            last = 4 * j + 3
            cn = cnp.tile([128, OT, QCH], BF16, tag="cn", name=f"cn{j}")
            state["cn", j] = cn
            # last chunk: hold back 2 fillers (same in-loop cadence) -
            # they drain post-loop, covering the PE while the last exps
            # finish before the epilogue's P@V pops.
            niter = (last + 1) * OT
            nfill = len(fillers) - (2 if j == NCH - 1 else 0)
            drained = 0
            it = 0
            for hp in range(OT):
                cx0 = ps.tile([128, QCH], F32, tag="cx", name="cx0")
                cx1 = ps.tile([128, QCH], F32, tag="cx", name="cx1")
                for i in range(last + 1):
                    s = 128 * (i - 4 * j) if i >= 4 * j else 0
                    st = ps.tile([128, 2, QCH], F32, tag="sc", name="st")
                    for hh in range(2):
                        hs = slice(HD * hh, HD * (hh + 1))
                        nc.tensor.matmul(
                            st[:, hh, s:QCH],
                            kt[hs, hp, bass.ts(i, 128)],
                            qt[hs, hp, j * QCH + s:(j + 1) * QCH],
                            start=True, stop=True)
                    pt = ptp.tile([128, 2, QCH], BF16, tag="pt")
                    pending.append((pt, s, i, hp, cx0, cx1, i == last, cn,
                                    j == NCH - 1 and hp == OT - 1))
                    nc.scalar.activation(pt[:, :, s:QCH], st[:, :, s:QCH], AF.Exp)
                    if i >= 4 * j:
                        for hh in range(2):
                            nc.vector.tensor_mul(
                                pt[:, hh, s:s + 128], pt[:, hh, s:s + 128],
                                cmask[:])
                    # lag-3 pop: exp(i) gets three iterations before its P@V
                    # is needed, absorbing ACT-queue jitter without a PE gap.
                    if len(pending) >= 4:
                        pop_pv()
                    if i == 3 and deferred:
                        deferred.pop(0)()  # prev head-pair's normalize
                    # drain fillers at a uniform rate over the chunk so the
                    # exp-paced tail iterations still get PE filler work.
                    it += 1
                    while fillers and drained < it * nfill // niter:
                        fillers.popleft()()
                        drained += 1

        # ---- main schedule ----
        emit_xdma(0)
        # prologue is DMA-paced: interleave the first two v tiles per-ktile
        # so the PE consumes weight/x pieces as they land.
        vdir_pair(0, 0, 1)()
        proj_item(0, wq, qt, 0)()
        proj_item(0, wk, kt, 0)()
        fillers.append(vdir_pair(0, 2, 3))
        for ot in range(1, OT):
            fillers.append(proj_item(0, wq, qt, ot))
            fillers.append(proj_item(0, wk, kt, ot))
        # ALL output projections are deferred to the last chunk: it has no
        # projection fillers and is otherwise exp-paced, so the po work
        # keeps the PE saturated (and its p-state high) there, while the
        # mid chunks stay PE-paced.
        for c in range(NCH):
            if c + 1 < NCH:
                emit_xdma(c + 1)
                push_chunk_fillers(c + 1)
            if c == NCH - 1:
                for cc in range(NCH - 1):
                    for t in range(4):
                        for dc in range(2):
                            fillers.append(po_item(cc, t, dc))
            attention_chunk(c)
            while fillers:
                fillers.popleft()()

        # ---- epilogue: every (t, dc) output chain runs k=0..2 as partials
        # overlapping the last head-pair's normalize; only the k=3 links
        # wait for it. ----
        j = NCH - 1
        cn = state["cn", j]

        while pending:               # final 3 P@V pairs + last norm recips
            pop_pv()
        for f in deferred:
            f()
        deferred.clear()
        po2a = ps.tile([128, 2, QCH], F32, tag="sc", name="pox0")
        for dc in range(2):
            for k in range(OT - 1):
                nc.tensor.matmul(po2a[:, dc, :], cn[:, k, bass.ts(0, 128)],
                                 wo[:, k, bass.ts(dc, QCH)],
                                 start=(k == 0), stop=False)
        po2b = ps.tile([128, 2, QCH], F32, tag="sc", name="pox1")
        for dc in range(2):
            for k in range(OT - 1):
                nc.tensor.matmul(po2b[:, dc, :], cn[:, k, bass.ts(1, 128)],
                                 wo[:, k, bass.ts(dc, QCH)],
                                 start=(k == 0), stop=False)
        part1 = []   # (po, t, dc) chains in cx slots: t=2
        for dc in range(2):
            po = ps.tile([128, QCH], F32, tag="cx", name="pox2")
            for k in range(OT - 1):
                nc.tensor.matmul(po[:], cn[:, k, bass.ts(2, 128)],
                                 wo[:, k, bass.ts(dc, QCH)],
                                 start=(k == 0), stop=False)
            part1.append((po, 2, dc))
        state["fnormb"]()            # PE rbt broadcasts into fl slots
        part3 = []   # t=3 partials in fl slots (rotation waits rbt muls)
        for dc in range(2):
            po = ps.tile([128, QCH], F32, tag="fl", name="pox3")
            for k in range(OT - 1):
                nc.tensor.matmul(po[:], cn[:, k, bass.ts(3, 128)],
                                 wo[:, k, bass.ts(dc, QCH)],
                                 start=(k == 0), stop=False)
            part3.append((po, 3, dc))
        # k=3 links split into K=64 halves: the lower half only needs the
        # DVE-written cn[0:64,3] while the upper half waits the ct1 DMA.
        finals = ([(po2a[:, dc, :], 0, dc) for dc in range(2)]
                  + [(po2b[:, dc, :], 1, dc) for dc in range(2)]
                  + [(po[:], t, dc) for po, t, dc in part1 + part3])
        for po_ap, t, dc in finals:
            nc.tensor.matmul(po_ap, cn[0:HD, OT - 1, bass.ts(t, 128)],
                             wo[0:HD, OT - 1, bass.ts(dc, QCH)],
                             start=False, stop=False)
        for po_ap, t, dc in finals:
            nc.tensor.matmul(po_ap, cn[HD:128, OT - 1, bass.ts(t, 128)],
                             wo[HD:128, OT - 1, bass.ts(dc, QCH)],
                             start=False, stop=True)
        rings = [nc.sync, nc.scalar]
        for n, (po2, t) in enumerate(((po2a, 0), (po2b, 1))):
            ob = opool.tile([128, 2, QCH], BF16, tag="ob2")
            # final copies split DVE/ACT so they drain in parallel
            if n % 2:
                nc.scalar.activation(ob[:], po2[:], AF.Copy)
            else:
                nc.vector.tensor_copy(ob[:], po2[:])
            rings[n % 2].dma_start(
                OUT[(4 * j + t) * 128:(4 * j + t + 1) * 128, :],
                ob[:].rearrange("p a b -> p (a b)"))
        for n, (po, t, dc) in enumerate(part1 + part3):
            ob = opool.tile([128, QCH], BF16, tag="ob")
            if n % 2:
                nc.scalar.activation(ob[:], po[:], AF.Copy)
            else:
                nc.vector.tensor_copy(ob[:], po[:])
            rings[n % 2].dma_start(
                OUT[(4 * j + t) * 128:(4 * j + t + 1) * 128,
                    bass.ts(dc, QCH)],
                ob[:])


_CACHE = {}


def _build():
    nc = bacc.Bacc("TRN2", target_bir_lowering=False, debug=False,
                   num_devices=NCORES)
    _emit(nc)
    nc.compile()
    return nc


def _in_maps(x, Wq, Wk, Wv, Wo):
    import ml_dtypes
    bf16 = ml_dtypes.bfloat16
    x = np.asarray(x, dtype=np.float32)
    Wq = np.asarray(Wq, dtype=np.float32)
    Wk = np.asarray(Wk, dtype=np.float32)
    Wv = np.asarray(Wv, dtype=np.float32)
    Wo = np.asarray(Wo, dtype=np.float32)

    cmask = np.triu(np.ones((128, 128), np.float32)).astype(bf16)

    # x[b] -> [128, KO, S]: [p, k, s] = x[b, s, k*128+p]
    xtb = [np.ascontiguousarray(
        x[b].T.reshape(KO, 128, S).transpose(1, 0, 2)).astype(bf16)
        for b in range(B)]

    def wslice(W, g, scale=1.0):
        # [p, k, ot, n] = W_g[ot*128+n, k*128+p]
        wg = (W[g * CW:(g + 1) * CW, :] * scale).astype(np.float32)
        return np.ascontiguousarray(
            wg.reshape(OT, 128, KO, 128).transpose(3, 2, 0, 1)).astype(bf16)

    def woslice(Wo, g):
        # [p, kt, d] = Wo[d, g*512 + kt*128 + p]
        wg = Wo[:, g * CW:(g + 1) * CW].astype(np.float32)
        return np.ascontiguousarray(
            wg.reshape(D, OT, 128).transpose(2, 1, 0)).astype(bf16)

    wmaps = []
    for g in range(G):
        wmaps.append({
            "WQT": wslice(Wq, g, scale=1.0 / SCALE),
            "WKT": wslice(Wk, g),
            "WVT": wslice(Wv, g),
            "WOT": woslice(Wo, g),
        })

    maps = []
    for c in range(NCORES):
        b, g = c // G, c % G
        m = {"XTB": xtb[b], "CMASK": cmask}
        m.update(wmaps[g])
        maps.append(m)
    return maps


def _run(x, Wq, Wk, Wv, Wo, bo, trace=False):
    nc = _CACHE.get("nc")
    if nc is None:
        nc = _CACHE["nc"] = _build()
    maps = _in_maps(x, Wq, Wk, Wv, Wo)
    res = run_bass_kernel_spmd(nc, maps, list(range(NCORES)), trace=trace)
    bo = np.asarray(bo, dtype=np.float32)
    out = np.empty((B, S, D), dtype=np.float32)
    for b in range(B):
        out[b] = (res.results[G * b]["OUT"].astype(np.float32)
                  + res.results[G * b + 1]["OUT"].astype(np.float32) + bo)
    return out, res


def kernel(x, Wq, Wk, Wv, Wo, bo):
    out, _ = _run(x, Wq, Wk, Wv, Wo, bo)
    return out


# revision 41
# speedup vs baseline: 1.0057x; 1.0057x over previous
"""Multi-head causal attention (B=4, S=2048, D=1024, H=16, HD=64) on 8 trn2 cores.

Sharding: batch x head-group. Core c handles batch b = c//2 and heads
g*8..(g+1)*8 where g = c%2 (512 projection dims). The host sums the two
partial output projections per batch and adds the bias.

All matmuls run in bf16 (fp32r lowers to fp32_mode=HIGH on HW at ~2.5
cycles/row; bf16 streams at 1 cycle/row), accumulation stays fp32 in PSUM.

Schedule notes (v2):
  - PSUM tags are split so filler matmuls (projections / output proj) no
    longer rotate through the score slots: scores keep true
    double-buffering (tag "sc", 2x2 banks), fillers use tag "fl" (2x1),
    ctx accumulators tag "cx" (2x1).
  - The P@V pipeline (pending deque) flows across head-pair boundaries:
    the trailing P@Vs of head-pair h drain during the first iterations of
    h+1, so the PE never waits for the last exps of a head-pair.
  - The softmax normalization broadcast (1/denominator across the 64
    head-dim partitions) runs on GpSimd (partition_broadcast) instead of
    K=1 PE matmuls.
  - Weight DMAs are split per-2-ktile pieces across idle rings (wv on
    scalar, wq on vector, wk on sync) so first projections start as the
    pieces land; wo/cmask ride gpsimd.
  - Output-projection DMA writes ride gpsimd (sw dge) instead of the
    Activation ring, keeping the exp engine clear.
  - Epilogue: 6 partial output chains (k=0..2) overlap the final
    normalize; only the k=3 links wait for the last head-pair.
"""

from collections import deque
from contextlib import ExitStack

import numpy as np

import concourse.bass as bass
import concourse.tile as tile
from concourse import bacc, mybir
from concourse.bass_utils import run_bass_kernel_spmd

F32 = mybir.dt.float32
BF16 = mybir.dt.bfloat16
AF = mybir.ActivationFunctionType

B, S, D, H = 4, 2048, 1024, 16
HD = D // H          # 64
SCALE = float(np.sqrt(HD))
NCORES = 8
G = 2                # head groups (cores per batch)
HPC = H // G         # heads per core = 8
CW = HPC * HD        # per-core projection width = 512
KO = D // 128        # 8 contraction subtiles
OT = CW // 128       # 4 projection out-tiles (head pairs)
QCH = 512            # q chunk
NQT = S // 128       # 16 kv tiles
NCH = S // QCH       # 4 q chunks


def _emit(nc):
    XTB = nc.dram_tensor("XTB", [128, KO, S], BF16, kind="ExternalInput").ap()
    WQT = nc.dram_tensor("WQT", [128, KO, OT, 128], BF16, kind="ExternalInput").ap()
    WKT = nc.dram_tensor("WKT", [128, KO, OT, 128], BF16, kind="ExternalInput").ap()
    WVT = nc.dram_tensor("WVT", [128, KO, OT, 128], BF16, kind="ExternalInput").ap()
    WOT = nc.dram_tensor("WOT", [128, OT, D], BF16, kind="ExternalInput").ap()
    CMASK = nc.dram_tensor("CMASK", [128, 128], BF16, kind="ExternalInput").ap()
    OUT = nc.dram_tensor("OUT", [S, D], BF16, kind="ExternalOutput").ap()

    with tile.TileContext(nc) as tc, ExitStack() as ctx, \
            nc.allow_low_precision(reason="bf16 attention pipeline"):
        consts = ctx.enter_context(tc.tile_pool(name="consts", bufs=1))
        xpool = ctx.enter_context(tc.tile_pool(name="xpool", bufs=2))
        qkv = ctx.enter_context(tc.tile_pool(name="qkv", bufs=1))
        ptp = ctx.enter_context(tc.tile_pool(name="ptp", bufs=12))
        rpool = ctx.enter_context(tc.tile_pool(name="rpool", bufs=6))
        cpool = ctx.enter_context(tc.tile_pool(name="cpool", bufs=2))
        cnp = ctx.enter_context(tc.tile_pool(name="cnp", bufs=4))
        opool = ctx.enter_context(tc.tile_pool(name="opool", bufs=6))
        ps = ctx.enter_context(tc.tile_pool(name="ps", bufs=2, space="PSUM"))

        wq = consts.tile([128, KO, OT, 128], BF16, tag="wq")
        wk = consts.tile([128, KO, OT, 128], BF16, tag="wk")
        wv = consts.tile([128, KO, OT, 128], BF16, tag="wv")
        wo = consts.tile([128, OT, D], BF16, tag="wo")
        cmask = consts.tile([128, 128], BF16, tag="cmask")

        qt = qkv.tile([128, OT, S], BF16, tag="qt")
        kt = qkv.tile([128, OT, S], BF16, tag="kt")
        # v natural: [kv, kvtile, hp, hh, 65] with a ones column at 64.
        v_sb = qkv.tile([128, NQT, OT, 2, HD + 1], BF16, tag="v")
        nc.vector.memset(v_sb[:, :, :, :, HD:HD + 1], 1.0)
        ones64 = consts.tile([128, HD], BF16, tag="ones64")
        nc.vector.memset(ones64[:], 1.0)

        # weight pieces split so projections start as they land (small first
        # pieces for the earliest matmuls, wq interleaved before the wv tail
        # so the q projection isn't starved); few enough issues that the
        # Activation queue clears before the first exp.
        for k2 in range(0, KO, 2):
            nc.scalar.dma_start(wv[:, k2:k2 + 2], WVT[:, k2:k2 + 2])
        for k4 in range(0, KO, 4):
            nc.scalar.dma_start(wq[:, k4:k4 + 4], WQT[:, k4:k4 + 4])
        nc.scalar.dma_start(wo[:], WOT[:])

        state = {}
        fillers = deque()
        pending = deque()  # (pt, s, i, hp, cx0, cx1, stop, cn)
        deferred = []

        def emit_xdma(c):
            xt = xpool.tile([128, KO, QCH], BF16, tag="xt", name=f"xt{c}")
            for k in range(KO):  # per-ktile pieces so proj can start early
                nc.sync.dma_start(xt[:, k, :], XTB[:, k, bass.ts(c, QCH)])
            state["xt", c] = xt
            if c == 0:
                # wk + cmask ride sync after chunk-0 x.
                nc.sync.dma_start(cmask[:], CMASK[:])
                for k2 in range(0, KO, 2):
                    nc.sync.dma_start(wk[:, k2:k2 + 2], WKT[:, k2:k2 + 2])

        def proj_item(c, w, dst, ot):
            def f():
                xt = state["xt", c]
                pp = ps.tile([128, QCH], F32, tag="fl", name="pp")
                for k in range(KO):
                    nc.tensor.matmul(pp[:], w[:, k, ot, :], xt[:, k, :],
                                     start=(k == 0), stop=(k == KO - 1))
                nc.vector.tensor_copy(dst[:, ot, bass.ts(c, QCH)], pp[:])
            return f

        def vdir_item(c, i):
            """V in natural [kv, outdim] layout directly: x-tile stationary,
            Wv moving - no PE transpose needed."""
            def f():
                xt = state["xt", c]
                pp = ps.tile([128, QCH], F32, tag="fl", name="vd")
                for k in range(KO):
                    nc.tensor.matmul(
                        pp[:], xt[:, k, (i % 4) * 128:(i % 4 + 1) * 128],
                        wv[:, k, :, :].rearrange("p o n -> p (o n)"),
                        start=(k == 0), stop=(k == KO - 1))
                nc.vector.tensor_copy(
                    v_sb[:, i, :, :, 0:HD],
                    pp[:].rearrange("p (o h d) -> p o h d", o=OT, h=2))
            return f

        def vdir_pair(c, i0, i1):
            """Two v tiles with the per-ktile matmuls interleaved, so the
            DMA-paced prologue streams weight/x pieces into the PE as they
            land instead of stalling one chain per piece."""
            def f():
                xt = state["xt", c]
                pp0 = ps.tile([128, QCH], F32, tag="fl", name="vp0")
                pp1 = ps.tile([128, QCH], F32, tag="fl", name="vp1")
                wvf = wv[:, :, :, :].rearrange("p k o n -> p k (o n)")
                for k in range(KO):
                    for i, pp in ((i0, pp0), (i1, pp1)):
                        nc.tensor.matmul(
                            pp[:], xt[:, k, (i % 4) * 128:(i % 4 + 1) * 128],
                            wvf[:, k, :],
                            start=(k == 0), stop=(k == KO - 1))
                for i, pp in ((i0, pp0), (i1, pp1)):
                    nc.vector.tensor_copy(
                        v_sb[:, i, :, :, 0:HD],
                        pp[:].rearrange("p (o h d) -> p o h d", o=OT, h=2))
            return f

        def po_item(j, t, dc):
            def f():
                cn = state["cn", j]
                po = ps.tile([128, QCH], F32, tag="fl", name="po")
                for k in range(OT):
                    nc.tensor.matmul(po[:], cn[:, k, bass.ts(t, 128)],
                                     wo[:, k, bass.ts(dc, QCH)],
                                     start=(k == 0), stop=(k == OT - 1))
                ob = opool.tile([128, QCH], BF16, tag="ob")
                nc.vector.tensor_copy(ob[:], po[:])
                # gpsimd ring: a data-dependent write issue must not sit in
                # front of the latency-critical norm DMAs (sync) or exps
                # (scalar) - in-order queues stall behind it.
                nc.gpsimd.dma_start(
                    OUT[(4 * j + t) * 128:(4 * j + t + 1) * 128,
                        bass.ts(dc, QCH)],
                    ob[:])
            return f

        def push_chunk_fillers(c):
            """proj+vtrans for chunk c (to drain during chunk c-1). V first
            so its transposes overlap the q/k projections in the prologue."""
            fillers.append(vdir_item(c, 4 * c))
            fillers.append(vdir_item(c, 4 * c + 1))
            fillers.append(proj_item(c, wq, qt, 0))
            fillers.append(proj_item(c, wk, kt, 0))
            fillers.append(vdir_item(c, 4 * c + 2))
            fillers.append(vdir_item(c, 4 * c + 3))
            for ot in range(1, OT):
                fillers.append(proj_item(c, wq, qt, ot))
                fillers.append(proj_item(c, wk, kt, ot))

        def norm_a(hp, cn, cx0, cx1, ring=None):
            """Copy ctx+denom to SBUF (frees the cx PSUM banks), compute
            1/d partition-parallel: the two [1,512] denominator rows are
            DMA-reshaped to [128,8], one cheap DVE reciprocal covers both
            heads, and the rows are DMA-restored for a gpsimd broadcast."""
            ring = ring or nc.sync
            ub0 = cpool.tile([HD + 1, QCH], F32, tag="ub0")
            ub1 = cpool.tile([HD + 1, QCH], F32, tag="ub1")
            nc.vector.tensor_copy(ub0[:], cx0[0:HD + 1, :])
            nc.vector.tensor_copy(ub1[:], cx1[0:HD + 1, :])
            dsq = rpool.tile([128, 8], F32, tag="dsq")
            ring.dma_start(dsq[:, 0:4], ub0[HD:HD + 1, :])
            ring.dma_start(dsq[:, 4:8], ub1[HD:HD + 1, :])
            dsr = rpool.tile([128, 8], BF16, tag="dsr")
            nc.vector.reciprocal(dsr[:], dsq[:])
            r0 = rpool.tile([1, QCH], BF16, tag="r0")
            r1 = rpool.tile([1, QCH], BF16, tag="r1")
            ring.dma_start(r0[:], dsr[:, 0:4])
            ring.dma_start(r1[:], dsr[:, 4:8])
            return ub0, ub1, r0, r1

        def norm_b(hp, cn, ub0, ub1, r0, r1, ring=None):
            ring2 = ring or nc.sync

            def f():
                rb0 = rpool.tile([HD, QCH], BF16, tag="rb0")
                rb1 = rpool.tile([HD, QCH], BF16, tag="rb1")
                nc.gpsimd.partition_broadcast(rb0[:], r0[:], channels=HD)
                nc.gpsimd.partition_broadcast(rb1[:], r1[:], channels=HD)
                ct1 = cpool.tile([HD, QCH], BF16, tag="ct1")
                nc.vector.tensor_mul(ct1[:], ub1[0:HD, :], rb1[:])
                ring2.dma_start(cn[HD:128, hp, :], ct1[:])
                nc.vector.tensor_mul(cn[0:HD, hp, :], ub0[0:HD, :], rb0[:])
            return f

        def norm_b_pe(cn, ub0, ub1, r0, r1):
            """Final head-pair: broadcast 1/d with K=1 PE matmuls (no gpsimd
            wakeup+serial-broadcast hop on the critical path)."""
            rbt1 = ps.tile([128, QCH], F32, tag="fl", name="rbt1")
            rbt0 = ps.tile([128, QCH], F32, tag="fl", name="rbt0")
            nc.tensor.matmul(rbt1[0:HD, :], ones64[0:1, 0:HD], r1[:],
                             start=True, stop=True)
            nc.tensor.matmul(rbt0[0:HD, :], ones64[0:1, 0:HD], r0[:],
                             start=True, stop=True)
            ct1 = cpool.tile([HD, QCH], BF16, tag="ct1")
            nc.vector.tensor_mul(ct1[:], ub1[0:HD, :], rbt1[0:HD, :])
            nc.scalar.dma_start(cn[HD:128, OT - 1, :], ct1[:])
            nc.vector.tensor_mul(cn[0:HD, OT - 1, :], ub0[0:HD, :],
                                 rbt0[0:HD, :])

        def pop_pv():
            ppt, s, pi, hp, cx0, cx1, stop, cn, final = pending.popleft()
            for hh, cx in ((0, cx0), (1, cx1)):
                nc.tensor.matmul(cx[0:HD + 1, s:QCH],
                                 v_sb[:, pi, hp, hh, :],
                                 ppt[:, hh, s:QCH],
                                 start=(pi == 0), stop=stop)
            if stop:
                if final:
                    # latency-critical: recips on the empty scalar ring; the
                    # PE broadcast half is emitted later (state["fnormb"]).
                    ub0, ub1, r0, r1 = norm_a(hp, cn, cx0, cx1,
                                              ring=nc.scalar)
                    state["fnormb"] = lambda: norm_b_pe(cn, ub0, ub1, r0, r1)
                else:
                    deferred.append(norm_b(
                        hp, cn, *norm_a(hp, cn, cx0, cx1)))

        class Pacer:
            """Uniform filler drain over a phase's iterations; can hold a
            few fillers back for the phase end."""
            def __init__(self, niter, hold=0):
                self.niter = niter
                self.nfill = len(fillers) - hold
                self.it = 0
                self.drained = 0

            def step(self):
                self.it += 1
                while fillers and self.drained < \
                        self.it * self.nfill // self.niter:
                    fillers.popleft()()
                    self.drained += 1

        def attention_hp(j, hp, pacer):
            """One head-pair's kv-tile sweep for q chunk j."""
            last = 4 * j + 3
            if ("cn", j) not in state:
                state["cn", j] = cnp.tile([128, OT, QCH], BF16, tag="cn",
                                          name=f"cn{j}")
            cn = state["cn", j]
            cx0 = ps.tile([128, QCH], F32, tag="cx", name="cx0")
            cx1 = ps.tile([128, QCH], F32, tag="cx", name="cx1")
            for i in range(last + 1):
                s = 128 * (i - 4 * j) if i >= 4 * j else 0
                st = ps.tile([128, 2, QCH], F32, tag="sc", name="st")
                for hh in range(2):
                    hs = slice(HD * hh, HD * (hh + 1))
                    nc.tensor.matmul(
                        st[:, hh, s:QCH],
                        kt[hs, hp, bass.ts(i, 128)],
                        qt[hs, hp, j * QCH + s:(j + 1) * QCH],
                        start=True, stop=True)
                pt = ptp.tile([128, 2, QCH], BF16, tag="pt")
                pending.append((pt, s, i, hp, cx0, cx1, i == last, cn,
                                j == NCH - 1 and hp == OT - 1))
                nc.scalar.activation(pt[:, :, s:QCH], st[:, :, s:QCH], AF.Exp)
                if i >= 4 * j:
                    for hh in range(2):
                        nc.vector.tensor_mul(
                            pt[:, hh, s:s + 128], pt[:, hh, s:s + 128],
                            cmask[:])
                # lag-3 pop: exp(i) gets three iterations before its P@V
                # is needed, absorbing ACT-queue jitter without a PE gap.
                if len(pending) >= 4:
                    pop_pv()
                if i == 3 and deferred:
                    deferred.pop(0)()  # prev head-pair's normalize
                pacer.step()

        # ---- main schedule ----
        emit_xdma(0)
        # prologue is DMA-paced: interleave the first two v tiles per-ktile
        # so the PE consumes weight/x pieces as they land.
        vdir_pair(0, 0, 1)()
        proj_item(0, wq, qt, 0)()
        proj_item(0, wk, kt, 0)()
        fillers.append(vdir_pair(0, 2, 3))
        for ot in range(1, OT):
            fillers.append(proj_item(0, wq, qt, ot))
            fillers.append(proj_item(0, wk, kt, ot))
        # ALL output projections are deferred to the last chunk: it has no
        # projection fillers and is otherwise exp-paced, so the po work
        # keeps the PE saturated (and its p-state high) there, while the
        # mid chunks stay PE-paced. Additionally the last chunk's FIRST
        # head-pair is led into the second-to-last chunk's phase, so its
        # exps drain during that chunk's ACT slack and the final chunk's
        # exp queue never backs up.
        for c in range(NCH):
            if c + 1 < NCH:
                emit_xdma(c + 1)
                push_chunk_fillers(c + 1)
            if c == NCH - 1:
                for cc in range(NCH - 1):
                    for t in range(4):
                        for dc in range(2):
                            fillers.append(po_item(cc, t, dc))
            if c == NCH - 2:
                units = [(c, hp) for hp in range(OT)] + [(c + 1, 0)]
            elif c == NCH - 1:
                units = [(c, hp) for hp in range(1, OT)]
            else:
                units = [(c, hp) for hp in range(OT)]
            pacer = Pacer(sum(4 * jj + 4 for jj, _ in units),
                          hold=2 if c == NCH - 1 else 0)
            for jj, hp in units:
                attention_hp(jj, hp, pacer)
            while fillers:
                fillers.popleft()()

        # ---- epilogue: every (t, dc) output chain runs k=0..2 as partials
        # overlapping the last head-pair's normalize; only the k=3 links
        # wait for it. ----
        j = NCH - 1
        cn = state["cn", j]

        while pending:               # final 3 P@V pairs + last norm recips
            pop_pv()
        for f in deferred:
            f()
        deferred.clear()
        po2a = ps.tile([128, 2, QCH], F32, tag="sc", name="pox0")
        for dc in range(2):
            for k in range(OT - 1):
                nc.tensor.matmul(po2a[:, dc, :], cn[:, k, bass.ts(0, 128)],
                                 wo[:, k, bass.ts(dc, QCH)],
                                 start=(k == 0), stop=False)
        po2b = ps.tile([128, 2, QCH], F32, tag="sc", name="pox1")
        for dc in range(2):
            for k in range(OT - 1):
                nc.tensor.matmul(po2b[:, dc, :], cn[:, k, bass.ts(1, 128)],
                                 wo[:, k, bass.ts(dc, QCH)],
                                 start=(k == 0), stop=False)
        part1 = []   # (po, t, dc) chains in cx slots: t=2
        for dc in range(2):
            po = ps.tile([128, QCH], F32, tag="cx", name="pox2")
            for k in range(OT - 1):
                nc.tensor.matmul(po[:], cn[:, k, bass.ts(2, 128)],
                                 wo[:, k, bass.ts(dc, QCH)],
                                 start=(k == 0), stop=False)
            part1.append((po, 2, dc))
        state["fnormb"]()            # PE rbt broadcasts into fl slots
        part3 = []   # t=3 partials in fl slots (rotation waits rbt muls)
        for dc in range(2):
            po = ps.tile([128, QCH], F32, tag="fl", name="pox3")
            for k in range(OT - 1):
                nc.tensor.matmul(po[:], cn[:, k, bass.ts(3, 128)],
                                 wo[:, k, bass.ts(dc, QCH)],
                                 start=(k == 0), stop=False)
            part3.append((po, 3, dc))
        # k=3 links split into K=64 halves: the lower half only needs the
        # DVE-written cn[0:64,3] while the upper half waits the ct1 DMA.
        finals = ([(po2a[:, dc, :], 0, dc) for dc in range(2)]
                  + [(po2b[:, dc, :], 1, dc) for dc in range(2)]
                  + [(po[:], t, dc) for po, t, dc in part1 + part3])
        for po_ap, t, dc in finals:
            nc.tensor.matmul(po_ap, cn[0:HD, OT - 1, bass.ts(t, 128)],
                             wo[0:HD, OT - 1, bass.ts(dc, QCH)],
                             start=False, stop=False)
        for po_ap, t, dc in finals:
            nc.tensor.matmul(po_ap, cn[HD:128, OT - 1, bass.ts(t, 128)],
                             wo[HD:128, OT - 1, bass.ts(dc, QCH)],
                             start=False, stop=True)
        rings = [nc.sync, nc.scalar]
        for n, (po2, t) in enumerate(((po2a, 0), (po2b, 1))):
            ob = opool.tile([128, 2, QCH], BF16, tag="ob2")
            # final copies split DVE/ACT so they drain in parallel
            if n % 2:
                nc.scalar.activation(ob[:], po2[:], AF.Copy)
            else:
                nc.vector.tensor_copy(ob[:], po2[:])
            rings[n % 2].dma_start(
                OUT[(4 * j + t) * 128:(4 * j + t + 1) * 128, :],
                ob[:].rearrange("p a b -> p (a b)"))
        for n, (po, t, dc) in enumerate(part1 + part3):
            ob = opool.tile([128, QCH], BF16, tag="ob")
            if n % 2:
                nc.scalar.activation(ob[:], po[:], AF.Copy)
            else:
                nc.vector.tensor_copy(ob[:], po[:])
            rings[n % 2].dma_start(
                OUT[(4 * j + t) * 128:(4 * j + t + 1) * 128,
                    bass.ts(dc, QCH)],
                ob[:])


_CACHE = {}


def _build():
    nc = bacc.Bacc("TRN2", target_bir_lowering=False, debug=False,
                   num_devices=NCORES)
    _emit(nc)
    nc.compile()
    return nc


def _in_maps(x, Wq, Wk, Wv, Wo):
    import ml_dtypes
    bf16 = ml_dtypes.bfloat16
    x = np.asarray(x, dtype=np.float32)
    Wq = np.asarray(Wq, dtype=np.float32)
    Wk = np.asarray(Wk, dtype=np.float32)
    Wv = np.asarray(Wv, dtype=np.float32)
    Wo = np.asarray(Wo, dtype=np.float32)

    cmask = np.triu(np.ones((128, 128), np.float32)).astype(bf16)

    # x[b] -> [128, KO, S]: [p, k, s] = x[b, s, k*128+p]
    xtb = [np.ascontiguousarray(
        x[b].T.reshape(KO, 128, S).transpose(1, 0, 2)).astype(bf16)
        for b in range(B)]

    def wslice(W, g, scale=1.0):
        # [p, k, ot, n] = W_g[ot*128+n, k*128+p]
        wg = (W[g * CW:(g + 1) * CW, :] * scale).astype(np.float32)
        return np.ascontiguousarray(
            wg.reshape(OT, 128, KO, 128).transpose(3, 2, 0, 1)).astype(bf16)

    def woslice(Wo, g):
        # [p, kt, d] = Wo[d, g*512 + kt*128 + p]
        wg = Wo[:, g * CW:(g + 1) * CW].astype(np.float32)
        return np.ascontiguousarray(
            wg.reshape(D, OT, 128).transpose(2, 1, 0)).astype(bf16)

    wmaps = []
    for g in range(G):
        wmaps.append({
            "WQT": wslice(Wq, g, scale=1.0 / SCALE),
            "WKT": wslice(Wk, g),
            "WVT": wslice(Wv, g),
            "WOT": woslice(Wo, g),
        })

    maps = []
    for c in range(NCORES):
        b, g = c // G, c % G
        m = {"XTB": xtb[b], "CMASK": cmask}
        m.update(wmaps[g])
        maps.append(m)
    return maps


def _run(x, Wq, Wk, Wv, Wo, bo, trace=False):
    nc = _CACHE.get("nc")
    if nc is None:
        nc = _CACHE["nc"] = _build()
    maps = _in_maps(x, Wq, Wk, Wv, Wo)
    res = run_bass_kernel_spmd(nc, maps, list(range(NCORES)), trace=trace)
    bo = np.asarray(bo, dtype=np.float32)
    out = np.empty((B, S, D), dtype=np.float32)
    for b in range(B):
        out[b] = (res.results[G * b]["OUT"].astype(np.float32)
                  + res.results[G * b + 1]["OUT"].astype(np.float32) + bo)
    return out, res


def kernel(x, Wq, Wk, Wv, Wo, bo):
    out, _ = _run(x, Wq, Wk, Wv, Wo, bo)
    return out
